# revision 6
# baseline (speedup 1.0000x reference)
"""Trainium2 Bass kernel for DenseRelativeLoc.

Computation (per batch b of 64):
  - gather 256 px-points and 256 py-points (columns of x[b] viewed as
    [C=768, HW=3136]) -> ptsT chunks [128c, 512s] via GPSIMD ap_gather
  - 3-layer MLP on the gathered features via TensorE matmuls in a
    transposed layout (activations kept as [feature-part, sample-free])
  - predxy [B*S, 2] written back; deltaxy computed host-side (pure
    integer arithmetic on the indices)

Sharding: data-parallel over batch, 8 batches per NeuronCore x 8 cores.
MLP weights replicated. No cross-core communication.
"""

import sys
import types
import contextlib
import ctypes

sys.path.insert(0, "/opt/trn_rl_repo")

import numpy as np

# ---------------------------------------------------------------- constants
B, C, H, W = 64, 768, 56, 56
HW = H * W            # 3136
S = 256               # points per batch (per side)
NIDX = 2 * S          # 512 gathered columns per batch (px then py)
NH = 256              # hidden width
OUT = 2
NCORES = 8
NB = B // NCORES      # batches per core = 8
KC = C // 128         # channel chunks = 6

_F32 = None           # set lazily (mybir dtype)
_PROGRAMS = {}        # cached compiled programs keyed by nb


def _install_ntff_hook():
    """Recreate antenv.axon_hooks (absent in this image) so that
    run_bass_kernel_spmd(trace=True) can register NTFF profiling."""
    import antenv

    if "antenv.axon_hooks" in sys.modules:
        return
    mod = types.ModuleType("antenv.axon_hooks")
    holder = {"hook": None}
    mod.set_axon_ntff_profile_hook = lambda h: holder.__setitem__("hook", h)
    mod.get_axon_ntff_profile_hook = lambda: holder["hook"]
    sys.modules["antenv.axon_hooks"] = mod
    antenv.axon_hooks = mod

    try:
        lib = ctypes.CDLL("/opt/axon/libaxon_pjrt.so")
    except OSError:
        return
    if not hasattr(lib, "axon_start_nrt_profile"):
        return
    lib.axon_start_nrt_profile.argtypes = [ctypes.POINTER(ctypes.c_int64), ctypes.c_size_t]
    lib.axon_start_nrt_profile.restype = ctypes.c_int64
    lib.axon_stop_nrt_profile.argtypes = [ctypes.c_char_p]
    lib.axon_stop_nrt_profile.restype = ctypes.c_int64

    @contextlib.contextmanager
    def _hook(output_dir, device_ids):
        import jax

        jax.devices()
        if device_ids:
            ids = (ctypes.c_int64 * len(device_ids))(*device_ids)
            rc = lib.axon_start_nrt_profile(ids, len(device_ids))
        else:
            rc = lib.axon_start_nrt_profile(None, 0)
        if rc != 0:
            raise RuntimeError(f"axon_start_nrt_profile rc={rc}")
        try:
            yield
        finally:
            n = lib.axon_stop_nrt_profile(str(output_dir).encode())
            print(f"profile: {n} file(s) written to {output_dir}", file=sys.stderr)

    mod.set_axon_ntff_profile_hook(_hook)


def build_program(nb=NB):
    """Build + compile the per-core Bass/Tile program (cached)."""
    global _F32
    if nb in _PROGRAMS:
        return _PROGRAMS[nb]

    import concourse.bass as bass
    import concourse.mybir as mybir
    import concourse.tile as tile
    from concourse import bacc
    from concourse.bass import ts

    f32 = mybir.dt.float32
    i16 = mybir.dt.int16
    _F32 = f32
    Relu = mybir.ActivationFunctionType.Relu

    nc = bacc.Bacc("TRN2", target_bir_lowering=False, debug=False, num_devices=NCORES)

    x_d = nc.dram_tensor("x", [nb, C, HW], f32, kind="ExternalInput")
    idx_d = nc.dram_tensor("idx", [128, nb, NIDX // 16], i16, kind="ExternalInput")
    w1_d = nc.dram_tensor("w1", [128, 2 * KC, NH], f32, kind="ExternalInput")
    w2_d = nc.dram_tensor("w2", [128, 2, NH], f32, kind="ExternalInput")
    w3_d = nc.dram_tensor("w3", [128, 2, OUT], f32, kind="ExternalInput")
    b1_d = nc.dram_tensor("b1", [128, 2], f32, kind="ExternalInput")
    b2_d = nc.dram_tensor("b2", [128, 2], f32, kind="ExternalInput")
    b3_d = nc.dram_tensor("b3", [128, OUT], f32, kind="ExternalInput")
    pred_d = nc.dram_tensor("pred", [128, nb, 2, OUT], f32, kind="ExternalOutput")

    with tile.TileContext(nc) as tc:
        with (
            tc.tile_pool(name="xp", bufs=4) as xp,
            tc.tile_pool(name="gp", bufs=8) as gp,
            tc.tile_pool(name="wp", bufs=1) as wp,
            tc.tile_pool(name="hp", bufs=3) as hp,
            tc.tile_pool(name="op", bufs=1) as op,
            tc.tile_pool(name="ps1", bufs=2, space="PSUM") as ps1,
            tc.tile_pool(name="ps2", bufs=2, space="PSUM") as ps2,
            tc.tile_pool(name="ps3", bufs=2, space="PSUM") as ps3,
        ):
            # ---- resident tensors
            w1t = wp.tile([128, 2 * KC, NH], f32, tag="w1")
            w2t = wp.tile([128, 2, NH], f32, tag="w2")
            w3t = wp.tile([128, 2, OUT], f32, tag="w3")
            b1t = wp.tile([128, 2], f32, tag="b1")
            b2t = wp.tile([128, 2], f32, tag="b2")
            b3t = wp.tile([128, OUT], f32, tag="b3")
            idxt = wp.tile([128, nb, NIDX // 16], i16, tag="idx")
            predt = op.tile([128, nb, 2, OUT], f32, tag="pred")

            nc.sync.dma_start(w1t[:], w1_d.ap())
            nc.sync.dma_start(w2t[:], w2_d.ap())
            nc.sync.dma_start(w3t[:], w3_d.ap())
            nc.sync.dma_start(b1t[:], b1_d.ap())
            nc.sync.dma_start(b2t[:], b2_d.ap())
            nc.sync.dma_start(b3t[:], b3_d.ap())
            nc.sync.dma_start(idxt[:], idx_d.ap())

            for b in range(nb):
                # ---- layer 1: hdn1T[n, s] += W1[c, n]^T-chunks @ ptsT[c, s]
                h1p = ps1.tile([128, 2, NH], f32, tag="h1p")
                gs = []
                for k in range(KC):
                    xt = xp.tile([128, HW], f32, tag="x")
                    nc.sync.dma_start(xt[:], x_d.ap()[b, ts(k, 128), :])
                    g = gp.tile([128, NIDX], f32, tag="g")
                    nc.gpsimd.ap_gather(
                        g[:], xt[:], idxt[:, b, :],
                        channels=128, num_elems=HW, d=1, num_idxs=NIDX,
                    )
                    gs.append(g)
                for nh in range(2):
                    for k in range(KC):
                        for xy in range(2):
                            nc.tensor.matmul(
                                h1p[:, nh, :],
                                w1t[:, xy * KC + k, ts(nh, 128)],
                                gs[k][:, ts(xy, S)],
                                start=(k == 0 and xy == 0),
                                stop=(k == KC - 1 and xy == 1),
                            )
                h1 = hp.tile([128, 2, NH], f32, tag="h1")
                for nh in range(2):
                    nc.scalar.activation(
                        h1[:, nh, :], h1p[:, nh, :], Relu,
                        bias=b1t[:, nh : nh + 1], scale=1.0,
                    )

                # ---- layer 2: hdn2T[m, s] = relu(W2[n, m]^T-chunks @ hdn1T[n, s] + b2)
                h2p = ps2.tile([128, 2, NH], f32, tag="h2p")
                for mh in range(2):
                    for nk in range(2):
                        nc.tensor.matmul(
                            h2p[:, mh, :],
                            w2t[:, nk, ts(mh, 128)],
                            h1[:, nk, :],
                            start=(nk == 0),
                            stop=(nk == 1),
                        )
                h2 = hp.tile([128, 2, NH], f32, tag="h2")
                for mh in range(2):
                    nc.scalar.activation(
                        h2[:, mh, :], h2p[:, mh, :], Relu,
                        bias=b2t[:, mh : mh + 1], scale=1.0,
                    )

                # ---- layer 3: pred[s, o] = hdn2T[m, s]^T-chunks @ W3[m, o] + b3
                pp = ps3.tile([128, 2, OUT], f32, tag="pp")
                for sh in range(2):
                    for mk in range(2):
                        nc.tensor.matmul(
                            pp[:, sh, :],
                            h2[:, mk, ts(sh, 128)],
                            w3t[:, mk, :],
                            start=(mk == 0),
                            stop=(mk == 1),
                        )
                for sh in range(2):
                    nc.vector.tensor_add(predt[:, b, sh, :], pp[:, sh, :], b3t[:])

            nc.sync.dma_start(pred_d.ap(), predt[:])

    nc.compile()
    from concourse.bass_interp import get_hw_module

    nc.m = get_hw_module(nc.m)
    _PROGRAMS[nb] = nc
    return nc


def _prep_core_inputs(x, pxs, pys, W1, b1, W2, b2, W3, b3):
    """Host-side shard + layout massage. Returns list of 8 in_maps."""
    x = np.asarray(x, dtype=np.float32).reshape(B, C, HW)
    pxs = np.asarray(pxs).astype(np.int64)
    pys = np.asarray(pys).astype(np.int64)

    # flat gather indices, int16, wrapped [16, NIDX/16] and replicated to 128 partitions
    xi = pxs[:, :, 0] * H + pxs[:, :, 1]          # [B, S]
    yi = pys[:, :, 0] * H + pys[:, :, 1]          # [B, S]
    allidx = np.concatenate([xi, yi], axis=1).astype(np.int16)  # [B, 512]
    wrapped = allidx.reshape(B, NIDX // 16, 16).transpose(0, 2, 1)  # [B, 16, 32]
    idx128 = np.tile(wrapped, (1, 8, 1)).reshape(B, 128, NIDX // 16)  # [B, 128, 32]

    w1k = np.ascontiguousarray(
        np.asarray(W1, dtype=np.float32).reshape(2 * KC, 128, NH).transpose(1, 0, 2)
    )
    w2k = np.ascontiguousarray(
        np.asarray(W2, dtype=np.float32).reshape(2, 128, NH).transpose(1, 0, 2)
    )
    w3k = np.ascontiguousarray(
        np.asarray(W3, dtype=np.float32).reshape(2, 128, OUT).transpose(1, 0, 2)
    )
    b1t = np.ascontiguousarray(np.asarray(b1, dtype=np.float32).reshape(2, 128).T)
    b2t = np.ascontiguousarray(np.asarray(b2, dtype=np.float32).reshape(2, 128).T)
    b3t = np.ascontiguousarray(
        np.broadcast_to(np.asarray(b3, dtype=np.float32), (128, OUT))
    )

    in_maps = []
    for c in range(NCORES):
        sl = slice(c * NB, (c + 1) * NB)
        in_maps.append(
            {
                "x": np.ascontiguousarray(x[sl]),
                "idx": np.ascontiguousarray(idx128[sl].transpose(1, 0, 2)),
                "w1": w1k,
                "w2": w2k,
                "w3": w3k,
                "b1": b1t,
                "b2": b2t,
                "b3": b3t,
            }
        )
    return in_maps


def _assemble_pred(results):
    """Per-core pred [128, NB, 2, 2] -> full predxy [B*S, 2]."""
    parts = []
    for c in range(NCORES):
        p = results[c]["pred"]  # [128, NB, 2, OUT]
        parts.append(np.ascontiguousarray(p.transpose(1, 2, 0, 3)).reshape(NB * 2 * 128, OUT))
    return np.concatenate(parts, axis=0)


def _run(inputs, trace=False):
    _install_ntff_hook()
    from concourse import bass_utils

    nc = build_program()
    in_maps = _prep_core_inputs(**inputs)
    res = bass_utils.run_bass_kernel_spmd(
        nc, in_maps, core_ids=list(range(NCORES)), trace=trace
    )
    predxy = _assemble_pred(res.results)

    pxs = np.asarray(inputs["pxs"]).astype(np.int64)
    pys = np.asarray(inputs["pys"]).astype(np.int64)
    deltaxy = (pxs - pys).astype(np.float32).reshape(-1, 2) + np.float32(H - 1)
    return (predxy, deltaxy), res


def kernel(**inputs):
    outs, _ = _run(inputs, trace=False)
    return outs
